# revision 1
# baseline (speedup 1.0000x reference)
"""GAT-style attention (gnn_message_passing) Trainium2 kernel, 8-core row-parallel.

Math (algebraically identical to the reference masked-softmax attention):
  E = relu(h @ P)                [N,3]
  W = exp(E)
  denom[i,k] = sum_j A[i,j] W[j,k]
  out[i,:]   = rowsum[i] * sum_k (1/denom[i,k]) * sum_j A[i,j] W[j,k] h[j,:]
             = rowsum[i] * ((A * C) @ h)[i,:],  C[i,j] = sum_k W[j,k]/denom[i,k]

Two SPMD programs (collectives are unavailable on this runtime path, so the
tiny [4096,3] W matrix crosses cores via a host gather between programs):
  P1 (per core): W-shard = max(exp(relu(h_shard @ P) - 4ln2), 1/16)  [512,3]
  host: concatenate the 8 W-shards -> W_full [4096,3]  (pure data movement)
  P2 (per core): load A-shard cast to fp16, xbar-transpose to A.T;
      denom via PE (W'|1 stationary, i-sliced to pipeline with the A load);
      C.T tiles via PE (K=3); mask-multiply on DVE; main (A*C).T @ h on PE
      with h streaming during the loop; scale by rowsum/1024.
W is pre-scaled by 2^-4 and R by 1024 so everything fits fp16 range.
"""

import numpy as np

import concourse.bass as bass
import concourse.mybir as mybir
import concourse.tile as tile
from concourse import bacc
from concourse import bass_utils

N = 4096
D = 512
H = 3
NCORES = 8
SH = N // NCORES          # 512 output rows per core
JC = N // 128             # 32 j-chunks
IC = SH // 128            # 4 i-chunks
DC = D // 128             # 4 d-chunks
F16 = mybir.dt.float16
F32 = mybir.dt.float32
LN2x4 = float(4.0 * np.log(2.0))   # W scaled by 2^-4 to stay in fp16 range
RSCALE = 1024.0                    # R' = 1024/denom; undone in the final scale


def _body1(tc, h_shard_t, p_in, id_in, w_out):
    """P1: W-shard [SH,3] from h_shard.T [D,SH] and P."""
    nc = tc.nc
    with (
        tc.tile_pool(name="sb1", bufs=1) as sb,
        tc.tile_pool(name="ps1", bufs=2, space="PSUM") as ps,
    ):
        hst = sb.tile([128, DC * SH], F16, tag="hst")
        p16 = sb.tile([128, DC * H], F16, tag="p16")
        wsT = sb.tile([3, SH], F16, tag="wsT")
        bc = sb.tile([128, 12], F16, tag="bc")
        id16 = sb.tile([128, 128], F16, tag="id16")
        ebias = sb.tile([3, 1], F32, tag="ebias")
        nc.sync.dma_start(id16[:], id_in)
        nc.vector.memset(ebias[:], -LN2x4)

        nc.gpsimd.dma_start(
            out=hst[:].rearrange("p (dc j) -> p dc j", j=SH),
            in_=h_shard_t.rearrange("(dc p) j -> p dc j", p=128),
        )
        nc.gpsimd.dma_start(
            out=p16[:].rearrange("p (dc k) -> p dc k", k=H),
            in_=p_in.rearrange("(dc p) k -> p dc k", p=128),
        )
        psE = ps.tile([3, SH], F32, tag="scr", name="psE")
        for dc in range(DC):
            nc.tensor.matmul(
                psE[:],
                p16[:, dc * H:(dc + 1) * H],
                hst[:, dc * SH:(dc + 1) * SH],
                start=(dc == 0),
                stop=(dc == DC - 1),
            )
        nc.scalar.activation(
            wsT[:], psE[:], mybir.ActivationFunctionType.Exp,
            bias=ebias[:], scale=1.0,
        )
        nc.vector.tensor_scalar_max(wsT[:], wsT[:], 0.0625)
        psW = ps.tile([128, 16], F16, tag="scr", name="psW")
        for t in range(4):
            nc.tensor.transpose(
                psW[:, t * 4:t * 4 + 3],
                wsT[:, t * 128:(t + 1) * 128],
                id16[0:3, 0:3],
            )
        nc.vector.tensor_copy(
            bc[:].rearrange("p (t k) -> p t k", k=3),
            psW[:].rearrange("p (t s) -> p t s", s=4)[:, :, 0:3],
        )
        nc.sync.dma_start(
            out=w_out.rearrange("(t p) k -> p t k", p=128),
            in_=bc[:].rearrange("p (t k) -> p t k", k=3),
        )


def _body2(tc, a_rows, h_full, wt_in, w4_in, id_in, repl_in, out):
    """P2: the heavy pipeline. wt_in [3,N] / w4_in [128,JC*4] are host-layouts
    of the device-computed (scaled) W from P1."""
    nc = tc.nc
    mult = mybir.AluOpType.mult

    with (
        tc.tile_pool(name="big", bufs=1) as big,
        tc.tile_pool(name="small", bufs=1) as small,
        tc.tile_pool(name="mtp", bufs=4) as mtp,
        tc.tile_pool(name="osb", bufs=2) as osb,
        tc.tile_pool(name="psa", bufs=4, space="PSUM") as psa,
        tc.tile_pool(name="pso", bufs=1, space="PSUM") as pso,
    ):
        h16 = big.tile([128, JC * D], F16, tag="h16")       # h, j on partitions
        a16 = big.tile([128, IC * N], F16, tag="a16")       # A-shard natural
        at16 = big.tile([128, JC * SH], F16, tag="at16")    # A-shard transposed
        wT4 = small.tile([128, (JC // 4) * 128], F16, tag="wT4")  # W.T 4-row-packed
        repl = small.tile([3, 128], F16, tag="repl")        # R replication mm
        w4 = small.tile([128, JC * 4], F16, tag="w4")       # W'|ones (j on part)
        rT = small.tile([3, SH], F32, tag="rT")             # 1/denom (f32)
        rT16 = small.tile([3, SH], F16, tag="rT16")         # R' = 1024/denom
        rT16r = small.tile([128, SH], F16, tag="rT16r")     # R' at partitions 32t
        dn = small.tile([4, SH], F16, tag="dn")             # denom.T staging
        rs4 = small.tile([128, IC * 4], F32, tag="rs4")     # rowsum per-partition
        id16 = small.tile([128, 128], F16, tag="id16")

        nc.sync.dma_start(id16[:], id_in)

        # ---------------- loads ----------------
        # A first at full bandwidth (SWDGE cast, split for pipelining);
        # W tiles are fp16 already - plain HWDGE loads on a parallel queue;
        # h afterwards - it streams during the main loop. A is transposed on
        # the PE (identity matmuls) so the DMA engines never switch xbar mode
        # (transpose<->copy transitions serialize the whole DMA pipeline).
        a_r = a_rows.rearrange("(ic p) j -> ic p j", p=128)
        for ic in range(IC):
            for hh in range(2):
                nc.gpsimd.dma_start(
                    out=a16[:, ic * N + hh * (N // 2): ic * N + (hh + 1) * (N // 2)],
                    in_=a_r[ic][:, hh * (N // 2):(hh + 1) * (N // 2)],
                )
        nc.sync.dma_start(out=wT4[:], in_=wt_in)
        nc.sync.dma_start(out=w4[:], in_=w4_in)
        nc.sync.dma_start(out=repl[:], in_=repl_in)

        h_r = h_full.rearrange("(g jc p) d -> g p jc d", p=128, jc=4)
        h16_v = h16[:].rearrange("p (jc d) -> p jc d", d=D)
        for g in range(8):
            nc.gpsimd.dma_start(
                out=h16_v[:, g * 4:(g + 1) * 4, :], in_=h_r[g]
            )

        # ------ A transpose on PE (8 tiles per PSUM bank, ACT/DVE copies) ----
        at_v = at16[:].rearrange("p (jc i) -> p jc i", i=SH)
        for ic in range(IC):
            for g in range(JC // 8):
                xp = psa.tile([128, 8 * 128], F16, tag="scr", name=f"xp{ic}_{g}")
                for t in range(8):
                    jc = 8 * g + t
                    nc.tensor.transpose(
                        xp[:, t * 128:(t + 1) * 128],
                        a16[:, ic * N + jc * 128: ic * N + (jc + 1) * 128],
                        id16[:],
                    )
                dst = at_v[:, 8 * g:8 * (g + 1), ic * 128:(ic + 1) * 128]
                srcv = xp[:].rearrange("p (t i) -> p t i", i=128)
                if (ic * 4 + g) % 2 == 0:
                    nc.scalar.copy(dst, srcv)
                else:
                    nc.vector.tensor_copy(dst, srcv)

        # ------------- denominators (i-sliced to pipeline with transposes) ----
        psD = psa.tile([4, SH], F32, tag="scr", name="psD")
        for ic in range(IC):
            for jc in range(JC):
                nc.tensor.matmul(
                    psD[:, ic * 128:(ic + 1) * 128],
                    w4[:, jc * 4:(jc + 1) * 4],
                    at16[:, jc * SH + ic * 128: jc * SH + ic * 128 + 128],
                    start=(jc == 0),
                    stop=(jc == JC - 1),
                )

        nc.vector.reciprocal(rT[:], psD[0:3, :])
        nc.vector.tensor_scalar_mul(rT16[:], rT[:], RSCALE)
        psRep = psa.tile([128, SH], F32, tag="scr", name="psRep")
        nc.tensor.matmul(psRep[:], repl[:], rT16[:], start=True, stop=True)
        nc.vector.tensor_copy(rT16r[:], psRep[:])
        nc.vector.tensor_copy(dn[:], psD[:])
        psR = psa.tile([128, 16], F16, tag="scr", name="psR")
        for t in range(4):
            nc.tensor.transpose(
                psR[:, t * 4:(t + 1) * 4], dn[:, t * 128:(t + 1) * 128],
                id16[0:4, 0:4],
            )
        nc.scalar.activation(
            rs4[:], psR[:], mybir.ActivationFunctionType.Copy,
            bias=0.0, scale=1.0 / RSCALE,
        )

        # ---------------- main loop ----------------
        psO = [
            pso.tile([128, D], F32, tag=f"psO{ic}", name=f"psO{ic}")
            for ic in range(IC)
        ]
        for g in range(JC // 4):
            cts = []
            for t in range(4):
                ct = psa.tile([128, SH], F32, tag="scr", name=f"ct{g}_{t}")
                nc.tensor.matmul(
                    ct[:],
                    wT4[32 * t:32 * t + 3, g * 128:(g + 1) * 128],
                    rT16r[32 * t:32 * t + 3, :],
                    start=True,
                    stop=True,
                    tile_position=(32 * t, 0),
                )
                cts.append(ct)
            for t in range(4):
                jc = 4 * g + t
                mt = mtp.tile([128, SH], F16, tag="mt", name=f"mt{jc}")
                nc.vector.tensor_tensor(
                    mt[:], at16[:, jc * SH:(jc + 1) * SH], cts[t][:], op=mult
                )
                for ic in range(IC):
                    nc.tensor.matmul(
                        psO[ic][:],
                        mt[:, ic * 128:(ic + 1) * 128],
                        h16[:, jc * D:(jc + 1) * D],
                        start=(jc == 0),
                        stop=(jc == JC - 1),
                    )

        # ---------------- scale + store ----------------
        out_r = out.rearrange("(ic p) d -> ic p d", p=128)
        for ic in range(IC):
            ot = osb.tile([128, D], F32, tag="ot")
            nc.vector.tensor_scalar(
                ot[:], psO[ic][:], rs4[:, 4 * ic + 3: 4 * ic + 4], None, op0=mult
            )
            nc.sync.dma_start(out=out_r[ic], in_=ot[:])


_CACHE = {}


def _build1():
    if "p1" in _CACHE:
        return _CACHE["p1"]
    nc = bacc.Bacc("TRN2", target_bir_lowering=False, debug=False,
                   num_devices=NCORES)
    h_shard_t = nc.dram_tensor("h_shard_t", [D, SH], F32,
                               kind="ExternalInput").ap()
    p_in = nc.dram_tensor("p_in", [D, H], F32, kind="ExternalInput").ap()
    id_in = nc.dram_tensor("id_in", [128, 128], F16, kind="ExternalInput").ap()
    w_out = nc.dram_tensor("w_out", [SH, H], F16, kind="ExternalOutput").ap()
    with tile.TileContext(nc) as tc:
        _body1(tc, h_shard_t, p_in, id_in, w_out)
    nc.compile()
    _CACHE["p1"] = nc
    return nc


def _build2():
    if "p2" in _CACHE:
        return _CACHE["p2"]
    nc = bacc.Bacc("TRN2", target_bir_lowering=False, debug=False,
                   num_devices=NCORES)
    a_rows = nc.dram_tensor("a_rows", [SH, N], F32, kind="ExternalInput").ap()
    h_full = nc.dram_tensor("h_full", [N, D], F32, kind="ExternalInput").ap()
    wt_in = nc.dram_tensor("wt_in", [128, (JC // 4) * 128], F16,
                          kind="ExternalInput").ap()
    w4_in = nc.dram_tensor("w4_in", [128, JC * 4], F16,
                           kind="ExternalInput").ap()
    id_in = nc.dram_tensor("id_in", [128, 128], F16, kind="ExternalInput").ap()
    repl_in = nc.dram_tensor("repl_in", [3, 128], F16,
                             kind="ExternalInput").ap()
    out = nc.dram_tensor("out", [SH, D], F32, kind="ExternalOutput").ap()
    with tile.TileContext(nc) as tc:
        _body2(tc, a_rows, h_full, wt_in, w4_in, id_in, repl_in, out)
    nc.compile()
    _CACHE["p2"] = nc
    return nc


def kernel(graph_info, h, P, _trace=False, _results_out=None):
    graph_info = np.ascontiguousarray(graph_info, dtype=np.float32)
    h = np.ascontiguousarray(h, dtype=np.float32)
    P = np.ascontiguousarray(P, dtype=np.float32)
    nc1 = _build1()
    nc2 = _build2()

    id_host = np.eye(128, dtype=np.float16)
    in1 = [
        {
            "h_shard_t": np.ascontiguousarray(h[c * SH:(c + 1) * SH, :].T),
            "p_in": P,
            "id_in": id_host,
        }
        for c in range(NCORES)
    ]
    res1 = bass_utils.run_bass_kernel_spmd(
        nc1, in1, core_ids=list(range(NCORES)), trace=_trace
    )
    w_full = np.concatenate(
        [res1.results[c]["w_out"] for c in range(NCORES)], axis=0
    )
    # wt4: W.T packed so 4 consecutive j-chunks sit in row groups 32t..32t+2
    wr = w_full.reshape(JC // 4, 4, 128, H)      # [g, t, i, k]
    wt_host = np.zeros((128, (JC // 4) * 128), np.float16)
    for t in range(4):
        for k in range(H):
            wt_host[32 * t + k, :] = wr[:, t, :, k].reshape(-1)
    repl_host = np.zeros((3, 128), np.float16)
    for t in range(4):
        for k in range(H):
            repl_host[k, 32 * t + k] = 1.0
    w4_host = np.concatenate(
        [w_full.reshape(JC, 128, H).transpose(1, 0, 2),
         np.ones((128, JC, 1), np.float16)],
        axis=2,
    ).reshape(128, JC * 4)
    w4_host = np.ascontiguousarray(w4_host)

    in2 = [
        {
            "a_rows": graph_info[c * SH:(c + 1) * SH, :],
            "h_full": h,
            "wt_in": wt_host,
            "w4_in": w4_host,
            "id_in": id_host,
            "repl_in": repl_host,
        }
        for c in range(NCORES)
    ]
    res2 = bass_utils.run_bass_kernel_spmd(
        nc2, in2, core_ids=list(range(NCORES)), trace=_trace
    )
    if _results_out is not None:
        _results_out.extend([res1, res2])
    return np.concatenate(
        [res2.results[c]["out"] for c in range(NCORES)], axis=0
    )



# revision 6
# speedup vs baseline: 1.2908x; 1.2908x over previous
"""GAT-style attention (gnn_message_passing) Trainium2 kernel, 8-core row-parallel.

Math (algebraically identical to the reference masked-softmax attention):
  E = relu(h @ P)                 [N,3]
  W' = max(exp(E - 4ln2), 1/16)   (= exp(relu(E))/16, fp16-safe range)
  denom'[i,k] = sum_j A[i,j] W'[j,k]   (k=3 slot sums ones -> rowsum[i])
  R'[i,k] = rowsum[i] / denom'[i,k]
  ct[j,i]  = sum_k W'[j,k] R'[i,k] = rowsum[i] * C[i,j]
  out[i,:] = sum_j A[i,j] ct[j,i] h[j,:]

Two SPMD programs (collectives unavailable on this runtime path; the tiny
[4096,3] W matrix crosses cores via a host gather between programs):
  P1 (per core): W'-shard [512,3] from host-transposed h-shard. The E matmuls
      use h.T as the *stationary* operand so each streams only 3 columns.
  host: concat the 8 W'-shards; pack W'.T, W'|ones; cast A-shard.T to fp8
      (binary, exact) and h to fp16  (pure data movement / layout).
  P2 (per core): denominators via at8-stationary matmuls ([128,4] outputs,
      accumulated in one PSUM bank); rowsum folded into R' so no final scale;
      C.T tiles via PE (K=3); mask-multiply on DVE; main (A*C).T @ h on PE
      with h streaming during the loop. PE warm-up matmuls run during the
      A.T load so the main loop starts at full clock.
"""

import numpy as np
import ml_dtypes

import concourse.bass as bass
import concourse.mybir as mybir
import concourse.tile as tile
from concourse import bacc
from concourse import bass_utils

N = 4096
D = 512
H = 3
NCORES = 8
SH = N // NCORES          # 512 output rows per core
JC = N // 128             # 32 j-chunks
IC = SH // 128            # 4 i-chunks
DC = D // 128             # 4 d-chunks
F8 = mybir.dt.float8e4
F16 = mybir.dt.float16
F32 = mybir.dt.float32
LN2x4 = float(4.0 * np.log(2.0))   # W scaled by 2^-4 to stay in fp16 range
N_WARMUP = 20                      # PE warm-up matmuls during the A.T load
NP_F8 = ml_dtypes.float8_e4m3


def _body1(tc, hst_in, p_in, w_out):
    """P1: W'-shard [SH,3] from hst [128, DC*SH] (h-shard.T, d on partitions)."""
    nc = tc.nc
    with (
        tc.tile_pool(name="sb1", bufs=1) as sb,
        tc.tile_pool(name="ps1", bufs=4, space="PSUM") as ps,
    ):
        hst = sb.tile([128, DC * SH], F16, tag="hst")
        p16 = sb.tile([128, DC * H], F16, tag="p16")
        wsE = sb.tile([128, IC * H], F16, tag="wsE")
        ebias = sb.tile([128, 1], F32, tag="ebias")
        nc.vector.memset(ebias[:], -LN2x4)
        nc.sync.dma_start(out=hst[:], in_=hst_in)
        nc.sync.dma_start(out=p16[:], in_=p_in)

        psE = [
            ps.tile([128, 512], F32, tag="scr", name=f"psE{jc}")
            for jc in range(IC)
        ]
        for jc in range(IC):
            for dc in range(DC):
                nc.tensor.matmul(
                    psE[jc][:, 0:H],
                    hst[:, dc * SH + jc * 128: dc * SH + (jc + 1) * 128],
                    p16[:, dc * H:(dc + 1) * H],
                    start=(dc == 0),
                    stop=(dc == DC - 1),
                )
        for jc in range(IC):
            nc.scalar.activation(
                wsE[:, jc * H:(jc + 1) * H], psE[jc][:, 0:H],
                mybir.ActivationFunctionType.Exp,
                bias=ebias[:], scale=1.0,
            )
        nc.vector.tensor_scalar_max(wsE[:], wsE[:], 0.0625)
        nc.sync.dma_start(
            out=w_out.rearrange("(jc p) k -> p jc k", p=128),
            in_=wsE[:].rearrange("p (jc k) -> p jc k", k=H),
        )


def _body2(tc, a8_in, h_in, wt_in, w4_in, id_in, out):
    """P2: the heavy pipeline. wt_in [3,N] / w4_in [128,JC*4] are host layouts
    of the device-computed (scaled) W' from P1; a8_in is A-shard.T in fp8."""
    nc = tc.nc
    mult = mybir.AluOpType.mult

    with (
        tc.tile_pool(name="big", bufs=1) as big,
        tc.tile_pool(name="small", bufs=1) as small,
        tc.tile_pool(name="mtp", bufs=4) as mtp,
        tc.tile_pool(name="osb", bufs=2) as osb,
        tc.tile_pool(name="psa", bufs=4, space="PSUM") as psa,
        tc.tile_pool(name="pso", bufs=1, space="PSUM") as pso,
    ):
        at8 = big.tile([128, JC * SH], F8, tag="at8")       # A.T, j on partitions
        h16 = big.tile([128, JC * D], F16, tag="h16")       # h, j on partitions
        wt = small.tile([3, N], F16, tag="wt")              # W'.T
        w4 = small.tile([128, JC * 4], F16, tag="w4")       # W'|ones (j on part)
        id16 = small.tile([128, 128], F16, tag="id16")
        scr = small.tile([128, 512], F16, tag="scr")        # warm-up source
        rN16 = small.tile([128, IC * H], F16, tag="rN16")   # rowsum/denom'
        rT16 = small.tile([3, SH], F16, tag="rT16")         # R'.T [k, i]

        # ---------------- loads ----------------
        # A.T (fp8, exact) first on the HWDGE/sync queue at full bandwidth;
        # h afterwards - it streams during the main loop. Small fp16 tiles on
        # the SWDGE/gpsimd queue so their generation doesn't delay A.T.
        a_r = a8_in.rearrange("(g jc p) i -> g p jc i", p=128, g=4)
        at8_v = at8[:].rearrange("p (g jc i) -> g p jc i", g=4, jc=JC // 4)
        for g in range(4):
            nc.sync.dma_start(out=at8_v[g], in_=a_r[g])
        h_r = h_in.rearrange("(g jc p) d -> g p jc d", p=128, g=8)
        h16_v = h16[:].rearrange("p (g jc d) -> g p jc d", g=8, jc=JC // 8)
        for g in range(8):
            nc.sync.dma_start(out=h16_v[g], in_=h_r[g])
        nc.gpsimd.dma_start(out=wt[:], in_=wt_in)
        nc.gpsimd.dma_start(out=w4[:], in_=w4_in)
        nc.gpsimd.dma_start(out=id16[:], in_=id_in)

        # ------ PE warm-up: ramp the clock while the A.T DMA is in flight ----
        nc.vector.memset(scr[:], 0.0)
        for w in range(N_WARMUP):
            pw = psa.tile([128, 512], F32, tag="scr", name=f"warm{w}")
            nc.tensor.matmul(
                pw[:], scr[:, 0:128], scr[:], start=True, stop=True
            )

        # ------------- denominators: at8 stationary, W' moving ----------
        # psD2[p_i, ic*4+k] = sum_j A[i,j] W'[j,k];  k=3 gives rowsum.
        # Groups run sequentially so a single PSUM bank serves all 4 ic.
        psD2 = psa.tile([128, IC * 4], F32, tag="scr", name="psD2")
        for ic in range(IC):
            for jc in range(JC):
                nc.tensor.matmul(
                    psD2[:, ic * 4:(ic + 1) * 4],
                    at8[:, jc * SH + ic * 128: jc * SH + ic * 128 + 128],
                    w4[:, jc * 4:(jc + 1) * 4],
                    start=(jc == 0),
                    stop=(jc == JC - 1),
                )

        # R' = rowsum/denom', transposed to [k, i] for the C matmuls
        psD2_v = psD2[:].rearrange("p (ic s) -> p ic s", s=4)
        for ic in range(IC):
            nc.vector.tensor_tensor(
                rN16[:, ic * H:(ic + 1) * H],
                psD2_v[:, ic, 3:4].broadcast_to((128, H)),
                psD2_v[:, ic, 0:H],
                op=mybir.AluOpType.divide,
            )
        psRT = psa.tile([3, SH], F16, tag="scr", name="psRT")
        for ic in range(IC):
            nc.tensor.transpose(
                psRT[:, ic * 128:(ic + 1) * 128],
                rN16[:, ic * H:(ic + 1) * H],
                id16[:],
            )
        nc.vector.tensor_copy(rT16[:], psRT[:])

        # ---------------- main loop ----------------
        psO = [
            pso.tile([128, D], F32, tag=f"psO{ic}", name=f"psO{ic}")
            for ic in range(IC)
        ]
        for jc in range(JC):
            ct = psa.tile([128, SH], F32, tag="scr", name=f"ct{jc}")
            nc.tensor.matmul(
                ct[:],
                wt[0:3, jc * 128:(jc + 1) * 128],
                rT16[:],
                start=True,
                stop=True,
                tile_position=(0, 0),
            )
            mt = mtp.tile([128, SH], F16, tag="mt", name=f"mt{jc}")
            nc.vector.tensor_tensor(
                mt[:], at8[:, jc * SH:(jc + 1) * SH], ct[:], op=mult
            )
            for ic in range(IC):
                nc.tensor.matmul(
                    psO[ic][:],
                    mt[:, ic * 128:(ic + 1) * 128],
                    h16[:, jc * D:(jc + 1) * D],
                    start=(jc == 0),
                    stop=(jc == JC - 1),
                )

        # ---------------- store ----------------
        out_r = out.rearrange("(ic p) d -> ic p d", p=128)
        for ic in range(IC):
            ot = osb.tile([128, D], F32, tag="ot")
            nc.scalar.copy(ot[:], psO[ic][:])
            nc.sync.dma_start(out=out_r[ic], in_=ot[:])


_CACHE = {}


def _build1():
    if "p1" in _CACHE:
        return _CACHE["p1"]
    nc = bacc.Bacc("TRN2", target_bir_lowering=False, debug=False,
                   num_devices=NCORES)
    hst_in = nc.dram_tensor("hst_in", [128, DC * SH], F16,
                            kind="ExternalInput").ap()
    p_in = nc.dram_tensor("p_in", [128, DC * H], F16, kind="ExternalInput").ap()
    w_out = nc.dram_tensor("w_out", [SH, H], F16, kind="ExternalOutput").ap()
    with tile.TileContext(nc) as tc:
        _body1(tc, hst_in, p_in, w_out)
    nc.compile()
    _CACHE["p1"] = nc
    return nc


def _build2():
    if "p2" in _CACHE:
        return _CACHE["p2"]
    nc = bacc.Bacc("TRN2", target_bir_lowering=False, debug=False,
                   num_devices=NCORES)
    a8_in = nc.dram_tensor("a8_in", [N, SH], F8, kind="ExternalInput").ap()
    h_in = nc.dram_tensor("h_in", [N, D], F16, kind="ExternalInput").ap()
    wt_in = nc.dram_tensor("wt_in", [3, N], F16, kind="ExternalInput").ap()
    w4_in = nc.dram_tensor("w4_in", [128, JC * 4], F16,
                           kind="ExternalInput").ap()
    id_in = nc.dram_tensor("id_in", [128, 128], F16, kind="ExternalInput").ap()
    out = nc.dram_tensor("out", [SH, D], F32, kind="ExternalOutput").ap()
    with tile.TileContext(nc) as tc:
        _body2(tc, a8_in, h_in, wt_in, w4_in, id_in, out)
    nc.compile()
    _CACHE["p2"] = nc
    return nc


def kernel(graph_info, h, P, _trace=False, _results_out=None):
    graph_info = np.ascontiguousarray(graph_info, dtype=np.float32)
    h = np.ascontiguousarray(h, dtype=np.float32)
    P = np.ascontiguousarray(P, dtype=np.float32)
    nc1 = _build1()
    nc2 = _build2()

    # host-side shard/layout prep (pure data movement + dtype casts)
    h16_full = h.astype(np.float16)
    p16_host = np.ascontiguousarray(
        P.astype(np.float16).reshape(DC, 128, H).transpose(1, 0, 2)
    ).reshape(128, DC * H)
    in1 = []
    for c in range(NCORES):
        hsT = h16_full[c * SH:(c + 1) * SH, :].T  # [D, SH]
        hst_host = np.ascontiguousarray(
            hsT.reshape(DC, 128, SH).transpose(1, 0, 2)
        ).reshape(128, DC * SH)
        in1.append({"hst_in": hst_host, "p_in": p16_host})
    res1 = bass_utils.run_bass_kernel_spmd(
        nc1, in1, core_ids=list(range(NCORES)), trace=_trace
    )
    w_full = np.concatenate(
        [res1.results[c]["w_out"] for c in range(NCORES)], axis=0
    )  # [N, 3] fp16, scaled by 2^-4

    wt_host = np.ascontiguousarray(w_full.T)  # [3, N]
    w4_host = np.ascontiguousarray(
        np.concatenate(
            [w_full.reshape(JC, 128, H).transpose(1, 0, 2),
             np.ones((128, JC, 1), np.float16)],
            axis=2,
        ).reshape(128, JC * 4)
    )
    id_host = np.eye(128, dtype=np.float16)

    in2 = [
        {
            "a8_in": np.ascontiguousarray(
                graph_info[c * SH:(c + 1) * SH, :].T
            ).astype(NP_F8),
            "h_in": h16_full,
            "wt_in": wt_host,
            "w4_in": w4_host,
            "id_in": id_host,
        }
        for c in range(NCORES)
    ]
    res2 = bass_utils.run_bass_kernel_spmd(
        nc2, in2, core_ids=list(range(NCORES)), trace=_trace
    )
    if _results_out is not None:
        _results_out.extend([res1, res2])
    return np.concatenate(
        [res2.results[c]["out"] for c in range(NCORES)], axis=0
    )


# revision 31
# speedup vs baseline: 1.4032x; 1.0871x over previous
"""GAT-style attention (gnn_message_passing) Trainium2 kernel, 8-core row-parallel.

Math (algebraically identical to the reference masked-softmax attention):
  E = relu(h @ P)                 [N,3]
  W' = max(exp(E - 4ln2), 1/16)   (= exp(relu(E))/16, fp16-safe range)
  denom'[i,k] = sum_j A[i,j] W'[j,k]   (k=3 slot sums ones -> rowsum[i])
  R'[i,k] = rowsum[i] / denom'[i,k]
  ct[j,i]  = sum_k W'[j,k] R'[i,k] = rowsum[i] * C[i,j]
  out[i,:] = sum_j A[i,j] ct[j,i] h[j,:]

Two SPMD programs (collectives unavailable on this runtime path; the tiny
[4096,3] W matrix crosses cores via a host gather between programs):
  P1 (per core): W'-shard [512,3] from host-transposed h-shard. The E matmuls
      use h.T as the *stationary* operand so each streams only 3 columns.
  host: concat the 8 W'-shards; pack W'.T, W'|ones; cast A-shard.T to fp8
      (binary, exact) and h to fp16  (pure data movement / layout).
  P2 (per core): denominators via at8-stationary matmuls ([128,4] outputs,
      accumulated in one PSUM bank); rowsum folded into R' so no final scale;
      C.T tiles via PE (K=3); mask-multiply on DVE; main (A*C).T @ h on PE
      with h streaming during the loop. PE warm-up matmuls run during the
      A.T load so the main loop starts at full clock.
"""

import numpy as np
import ml_dtypes

import concourse.bass as bass
import concourse.mybir as mybir
import concourse.tile as tile
from concourse import bacc
from concourse import bass_utils

N = 4096
D = 512
H = 3
NCORES = 8
SH = N // NCORES          # 512 output rows per core
JC = N // 128             # 32 j-chunks
IC = SH // 128            # 4 i-chunks
DC = D // 128             # 4 d-chunks
F8 = mybir.dt.float8e4
F16 = mybir.dt.float16
F32 = mybir.dt.float32
LN2x4 = float(4.0 * np.log(2.0))   # W scaled by 2^-4 to stay in fp16 range
N_WARMUP = 24                      # PE warm-up matmuls during the A.T load
NP_F8 = ml_dtypes.float8_e4m3
BCAST_DIV = False                  # stride-0 divide (walrus-compile suspect)


def _body1(tc, hst_in, p_in, w_out):
    """P1: W'-shard [SH,3] from hst [128, IC*DC*128] (h-shard.T, jc-major:
    hst[:, jc, dc, :] = h.T d-chunk dc for j-chunk jc). Loaded in 4 jc
    pieces; exp/max pipeline behind the pieces."""
    nc = tc.nc
    with (
        tc.tile_pool(name="sb1", bufs=1) as sb,
        tc.tile_pool(name="ps1", bufs=4, space="PSUM") as ps,
    ):
        hst = sb.tile([128, IC * DC * 128], F16, tag="hst")
        p16 = sb.tile([128, DC * H], F16, tag="p16")
        wsE = sb.tile([128, IC * H], F16, tag="wsE")
        ebias = sb.tile([128, 1], F32, tag="ebias")
        nc.vector.memset(ebias[:], -LN2x4)
        hst_v = hst[:].rearrange("p (g x) -> g p x", g=2)
        hin_v = hst_in.rearrange("p (g x) -> g p x", g=2)
        for g in range(2):
            nc.sync.dma_start(out=hst_v[g], in_=hin_v[g])
        nc.gpsimd.dma_start(out=p16[:], in_=p_in)

        # one PSUM tile spanning 4 banks: E group per jc, single exp at the end
        psE = ps.tile([128, IC * 512], F32, tag="psE", name="psE")
        for jc in range(IC):
            for dc in range(DC):
                nc.tensor.matmul(
                    psE[:, jc * 512: jc * 512 + H],
                    hst[:, (jc * DC + dc) * 128: (jc * DC + dc + 1) * 128],
                    p16[:, dc * H:(dc + 1) * H],
                    start=(dc == 0),
                    stop=(dc == DC - 1),
                )
        nc.scalar.activation(
            wsE[:].rearrange("p (jc k) -> p jc k", k=H),
            psE[:].rearrange("p (jc x) -> p jc x", x=512)[:, :, 0:H],
            mybir.ActivationFunctionType.Exp,
            bias=ebias[:], scale=1.0,
        )
        nc.vector.tensor_scalar_max(wsE[:], wsE[:], 0.0625)
        nc.sync.dma_start(out=w_out, in_=wsE[:])


def _body2(tc, a8_in, h_in, wt_in, w4_in, id_in, out):
    """P2: the heavy pipeline. wt_in [3,N] / w4_in [128,JC*4] are host layouts
    of the device-computed (scaled) W' from P1; a8_in is A-shard.T in fp8."""
    nc = tc.nc
    mult = mybir.AluOpType.mult

    with (
        tc.tile_pool(name="big", bufs=1) as big,
        tc.tile_pool(name="small", bufs=1) as small,
        tc.tile_pool(name="mtp", bufs=4) as mtp,
        tc.tile_pool(name="osb", bufs=4) as osb,
        tc.tile_pool(name="psa", bufs=3, space="PSUM") as psa,
        tc.tile_pool(name="psd", bufs=1, space="PSUM") as psd,
        tc.tile_pool(name="pso", bufs=1, space="PSUM") as pso,
    ):
        at8 = big.tile([128, JC * SH], F8, tag="at8")       # A.T, j on partitions
        h16 = big.tile([128, JC * D], F16, tag="h16")       # h, j on partitions
        wt = small.tile([3, N], F16, tag="wt")              # W'.T
        w4 = small.tile([128, JC * 4], F16, tag="w4")       # W'|ones (j on part)
        id16 = small.tile([128, 128], F16, tag="id16")
        scr = small.tile([128, 512], F16, tag="scr")        # warm-up source
        rN16 = small.tile([128, IC * H], F16, tag="rN16")   # rowsum/denom'
        rT16 = small.tile([3, SH], F16, tag="rT16")         # R'.T [k, i]

        # ---------------- loads ----------------
        # A.T (fp8, exact) first on the HWDGE/sync queue at full bandwidth;
        # h afterwards - it streams during the main loop. Small fp16 tiles on
        # the SWDGE/gpsimd queue so their generation doesn't delay A.T.
        a_r = a8_in.rearrange("(g jc p) i -> g p jc i", p=128, g=4)
        at8_v = at8[:].rearrange("p (g jc i) -> g p jc i", g=4, jc=JC // 4)
        for g in range(4):
            nc.sync.dma_start(out=at8_v[g], in_=a_r[g])
        h_r = h_in.rearrange("(g jc p) d -> g p jc d", p=128, g=8)
        h16_v = h16[:].rearrange("p (g jc d) -> g p jc d", g=8, jc=JC // 8)
        for g in range(8):
            nc.sync.dma_start(out=h16_v[g], in_=h_r[g])
        nc.gpsimd.dma_start(out=w4[:], in_=w4_in)
        nc.gpsimd.dma_start(out=id16[:], in_=id_in)
        nc.gpsimd.dma_start(out=wt[:], in_=wt_in)

        # ---- denominators chunked per A.T piece, PE warm-up filling gaps ----
        # psD2[p_i, ic*4+k] = sum_j A[i,j] W'[j,k];  k=3 gives rowsum.
        # The 4 ic accumulation regions share one PSUM bank (one group each,
        # started on the first piece, stopped on the last).
        nc.vector.memset(scr[:], 0.0)
        # warm the ACT table (LoadActFuncSet) off the critical path
        actw = small.tile([1, 2], F16, tag="actw")
        nc.scalar.copy(actw[:], scr[0:1, 0:2])
        n_warm = 0

        def warm(n):
            nonlocal n_warm
            for _ in range(n):
                pw = psa.tile([128, 512], F32, tag="scr", name=f"warm{n_warm}")
                nc.tensor.matmul(
                    pw[:], scr[:, 0:128], scr[:], start=True, stop=True
                )
                n_warm += 1

        psD2 = psd.tile([128, IC * 4], F32, tag="psd", name="psD2")
        warm(5)
        for g in range(4):
            for ic in range(IC):
                for jc in range(g * 8, (g + 1) * 8):
                    nc.tensor.matmul(
                        psD2[:, ic * 4:(ic + 1) * 4],
                        at8[:, jc * SH + ic * 128: jc * SH + ic * 128 + 128],
                        w4[:, jc * 4:(jc + 1) * 4],
                        start=(g == 0 and ic == 0 and jc == 0),
                        stop=(g == 3 and ic == IC - 1 and jc == 31),
                    )
            if g < 3:
                warm(7)

        # R' = rowsum/denom' in one DVE divide, transposed to [k, i] on PE
        psD2_v = psD2[:].rearrange("p (ic s) -> p ic s", s=4)
        if BCAST_DIV:
            nc.vector.tensor_tensor(
                rN16[:].rearrange("p (ic k) -> p ic k", k=H),
                psD2_v[:, :, 3:4].broadcast_to((128, IC, H)),
                psD2_v[:, :, 0:H],
                op=mybir.AluOpType.divide,
            )
        else:
            rN = small.tile([128, IC * H], F32, tag="rN")
            rsum = small.tile([128, IC], F32, tag="rsum")
            nc.vector.reciprocal(
                rN[:].rearrange("p (ic k) -> p ic k", k=H), psD2_v[:, :, 0:H]
            )
            nc.vector.tensor_copy(
                rsum[:].rearrange("p (ic o) -> p ic o", o=1), psD2_v[:, :, 3:4]
            )
            for ic in range(IC):
                nc.vector.tensor_scalar(
                    rN16[:, ic * H:(ic + 1) * H], rN[:, ic * H:(ic + 1) * H],
                    rsum[:, ic:ic + 1], None, op0=mult,
                )
        psRT = psa.tile([3, SH], F16, tag="scr", name="psRT")
        for ic in range(IC):
            nc.tensor.transpose(
                psRT[:, ic * 128:(ic + 1) * 128],
                rN16[:, ic * H:(ic + 1) * H],
                id16[:],
            )
        nc.vector.tensor_copy(rT16[:], psRT[:])

        # ---------------- main loop (ct software-pipelined 2 ahead) ----------
        psO = [
            pso.tile([128, D], F32, tag=f"psO{ic}", name=f"psO{ic}")
            for ic in range(IC)
        ]
        cts = [None] * JC

        def emit_ct(j):
            cts[j] = psa.tile([128, SH], F32, tag="scr", name=f"ct{j}")
            nc.tensor.matmul(
                cts[j][:],
                wt[0:3, j * 128:(j + 1) * 128],
                rT16[:],
                start=True,
                stop=True,
                tile_position=(0, 0),
            )

        emit_ct(0)
        emit_ct(1)
        for jc in range(JC):
            mt = mtp.tile([128, SH], F16, tag="mt", name=f"mt{jc}")
            nc.vector.tensor_tensor(
                mt[:], at8[:, jc * SH:(jc + 1) * SH], cts[jc][:], op=mult
            )
            if jc + 2 < JC:
                emit_ct(jc + 2)
            for ic in range(IC):
                nc.tensor.matmul(
                    psO[ic][:],
                    mt[:, ic * 128:(ic + 1) * 128],
                    h16[:, jc * D:(jc + 1) * D],
                    start=(jc == 0),
                    stop=(jc == JC - 1),
                )

        # ------- store: fp16 staging (host casts back), ACT/DVE copy split,
        # ------- DMAs split across HWDGE (sync) and SWDGE (gpsimd) queues ---
        out_r = out.rearrange("(ic p) d -> ic p d", p=128)
        for ic in range(IC):
            ot = osb.tile([128, D], F16, tag="ot", name=f"ot{ic}")
            if ic % 2 == 0:
                nc.scalar.copy(ot[:], psO[ic][:])
                nc.sync.dma_start(out=out_r[ic], in_=ot[:])
            else:
                nc.vector.tensor_copy(ot[:], psO[ic][:])
                nc.gpsimd.dma_start(out=out_r[ic], in_=ot[:])


_CACHE = {}


def _build1():
    if "p1" in _CACHE:
        return _CACHE["p1"]
    nc = bacc.Bacc("TRN2", target_bir_lowering=False, debug=False,
                   num_devices=NCORES)
    hst_in = nc.dram_tensor("hst_in", [128, IC * DC * 128], F16,
                            kind="ExternalInput").ap()
    p_in = nc.dram_tensor("p_in", [128, DC * H], F16, kind="ExternalInput").ap()
    w_out = nc.dram_tensor("w_out", [128, IC * H], F16,
                           kind="ExternalOutput").ap()
    with tile.TileContext(nc) as tc:
        _body1(tc, hst_in, p_in, w_out)
    nc.compile()
    _CACHE["p1"] = nc
    return nc


def _build2():
    if "p2" in _CACHE:
        return _CACHE["p2"]
    nc = bacc.Bacc("TRN2", target_bir_lowering=False, debug=False,
                   num_devices=NCORES)
    a8_in = nc.dram_tensor("a8_in", [N, SH], F8, kind="ExternalInput").ap()
    h_in = nc.dram_tensor("h_in", [N, D], F16, kind="ExternalInput").ap()
    wt_in = nc.dram_tensor("wt_in", [3, N], F16, kind="ExternalInput").ap()
    w4_in = nc.dram_tensor("w4_in", [128, JC * 4], F16,
                           kind="ExternalInput").ap()
    id_in = nc.dram_tensor("id_in", [128, 128], F16, kind="ExternalInput").ap()
    out = nc.dram_tensor("out", [SH, D], F16, kind="ExternalOutput").ap()
    with tile.TileContext(nc) as tc:
        _body2(tc, a8_in, h_in, wt_in, w4_in, id_in, out)
    nc.compile()
    _CACHE["p2"] = nc
    return nc


def kernel(graph_info, h, P, _trace=False, _results_out=None):
    graph_info = np.ascontiguousarray(graph_info, dtype=np.float32)
    h = np.ascontiguousarray(h, dtype=np.float32)
    P = np.ascontiguousarray(P, dtype=np.float32)
    nc1 = _build1()
    nc2 = _build2()

    # host-side shard/layout prep (pure data movement + dtype casts)
    h16_full = h.astype(np.float16)
    p16_host = np.ascontiguousarray(
        P.astype(np.float16).reshape(DC, 128, H).transpose(1, 0, 2)
    ).reshape(128, DC * H)
    in1 = []
    for c in range(NCORES):
        hsT = h16_full[c * SH:(c + 1) * SH, :].T  # [D, SH]
        hst_host = np.ascontiguousarray(
            hsT.reshape(DC, 128, IC, 128).transpose(1, 2, 0, 3)
        ).reshape(128, IC * DC * 128)
        in1.append({"hst_in": hst_host, "p_in": p16_host})
    res1 = bass_utils.run_bass_kernel_spmd(
        nc1, in1, core_ids=list(range(NCORES)), trace=_trace
    )
    w_full = np.concatenate(
        [
            res1.results[c]["w_out"]
            .reshape(128, IC, H).transpose(1, 0, 2).reshape(SH, H)
            for c in range(NCORES)
        ],
        axis=0,
    )  # [N, 3] fp16, scaled by 2^-4

    wt_host = np.ascontiguousarray(w_full.T)  # [3, N]
    w4_host = np.ascontiguousarray(
        np.concatenate(
            [w_full.reshape(JC, 128, H).transpose(1, 0, 2),
             np.ones((128, JC, 1), np.float16)],
            axis=2,
        ).reshape(128, JC * 4)
    )
    id_host = np.eye(128, dtype=np.float16)

    in2 = [
        {
            "a8_in": np.ascontiguousarray(
                graph_info[c * SH:(c + 1) * SH, :].T
            ).astype(NP_F8),
            "h_in": h16_full,
            "wt_in": wt_host,
            "w4_in": w4_host,
            "id_in": id_host,
        }
        for c in range(NCORES)
    ]
    res2 = bass_utils.run_bass_kernel_spmd(
        nc2, in2, core_ids=list(range(NCORES)), trace=_trace
    )
    if _results_out is not None:
        _results_out.extend([res1, res2])
    return np.concatenate(
        [res2.results[c]["out"].astype(np.float32) for c in range(NCORES)],
        axis=0,
    )


# revision 38
# speedup vs baseline: 1.4062x; 1.0022x over previous
"""GAT-style attention (gnn_message_passing) Trainium2 kernel, 8-core row-parallel.

Math (algebraically identical to the reference masked-softmax attention):
  E = relu(h @ P)                 [N,3]
  W' = max(exp(E - 4ln2), 1/16)   (= exp(relu(E))/16, fp16-safe range)
  denom'[i,k] = sum_j A[i,j] W'[j,k]   (k=3 slot sums ones -> rowsum[i])
  R'[i,k] = rowsum[i] / denom'[i,k]
  ct[j,i]  = sum_k W'[j,k] R'[i,k] = rowsum[i] * C[i,j]
  out[i,:] = sum_j A[i,j] ct[j,i] h[j,:]

Two SPMD programs (collectives unavailable on this runtime path; the tiny
[4096,3] W matrix crosses cores via a host gather between programs):
  P1 (per core): W'-shard [512,3] from host-transposed h-shard. The E matmuls
      use h.T as the *stationary* operand so each streams only 3 columns.
  host: concat the 8 W'-shards; pack W'.T, W'|ones; cast A-shard.T to fp8
      (binary, exact) and h to fp16  (pure data movement / layout).
  P2 (per core): denominators via at8-stationary matmuls ([128,4] outputs,
      accumulated in one PSUM bank); rowsum folded into R' so no final scale;
      C.T tiles via PE (K=3); mask-multiply on DVE; main (A*C).T @ h on PE
      with h streaming during the loop. PE warm-up matmuls run during the
      A.T load so the main loop starts at full clock.
"""

import numpy as np
import ml_dtypes

import concourse.bass as bass
import concourse.mybir as mybir
import concourse.tile as tile
from concourse import bacc
from concourse import bass_utils

N = 4096
D = 512
H = 3
NCORES = 8
SH = N // NCORES          # 512 output rows per core
JC = N // 128             # 32 j-chunks
IC = SH // 128            # 4 i-chunks
DC = D // 128             # 4 d-chunks
F8 = mybir.dt.float8e4
F16 = mybir.dt.float16
F32 = mybir.dt.float32
LN2x4 = float(4.0 * np.log(2.0))   # W scaled by 2^-4 to stay in fp16 range
N_WARMUP = 24                      # PE warm-up matmuls during the A.T load
NP_F8 = ml_dtypes.float8_e4m3
BCAST_DIV = False                  # stride-0 divide (walrus-compile suspect)


def _body1(tc, hst_in, p_in, w_out):
    """P1: W'-shard [SH,3] from hst [128, IC*DC*128] (h-shard.T, jc-major:
    hst[:, jc, dc, :] = h.T d-chunk dc for j-chunk jc). Loaded in 4 jc
    pieces; exp/max pipeline behind the pieces."""
    nc = tc.nc
    with (
        tc.tile_pool(name="sb1", bufs=1) as sb,
        tc.tile_pool(name="ps1", bufs=1, space="PSUM") as ps,
    ):
        hst = sb.tile([128, IC * DC * 128], F16, tag="hst")
        p16 = sb.tile([128, DC * H], F16, tag="p16")
        wsE = sb.tile([128, IC * H], F16, tag="wsE")
        ebias = sb.tile([128, 1], F32, tag="ebias")
        nc.vector.memset(ebias[:], -LN2x4)
        hst_v = hst[:].rearrange("p (g x) -> g p x", g=2)
        hin_v = hst_in.rearrange("p (g x) -> g p x", g=2)
        for g in range(2):
            nc.sync.dma_start(out=hst_v[g], in_=hin_v[g])
        nc.gpsimd.dma_start(out=p16[:], in_=p_in)

        # one PSUM tile spanning 4 banks: E group per jc, single exp at the end
        psE = ps.tile([128, IC * 512], F32, tag="psE", name="psE")
        for jc in range(IC):
            for dc in range(DC):
                nc.tensor.matmul(
                    psE[:, jc * 512: jc * 512 + H],
                    hst[:, (jc * DC + dc) * 128: (jc * DC + dc + 1) * 128],
                    p16[:, dc * H:(dc + 1) * H],
                    start=(dc == 0),
                    stop=(dc == DC - 1),
                )
        nc.scalar.activation(
            wsE[:].rearrange("p (jc k) -> p jc k", k=H),
            psE[:].rearrange("p (jc x) -> p jc x", x=512)[:, :, 0:H],
            mybir.ActivationFunctionType.Exp,
            bias=ebias[:], scale=1.0,
        )
        nc.vector.tensor_scalar_max(wsE[:], wsE[:], 0.0625)
        nc.sync.dma_start(out=w_out, in_=wsE[:])


def _body2(tc, a8_in, h_in, wt_in, w4_in, id_in, out):
    """P2: the heavy pipeline. wt_in [3,N] / w4_in [128,JC*4] are host layouts
    of the device-computed (scaled) W' from P1; a8_in is A-shard.T in fp8."""
    nc = tc.nc
    mult = mybir.AluOpType.mult

    with (
        tc.tile_pool(name="big", bufs=1) as big,
        tc.tile_pool(name="small", bufs=1) as small,
        tc.tile_pool(name="mtp", bufs=4) as mtp,
        tc.tile_pool(name="osb", bufs=4) as osb,
        tc.tile_pool(name="psa", bufs=3, space="PSUM") as psa,
        tc.tile_pool(name="psd", bufs=1, space="PSUM") as psd,
        tc.tile_pool(name="pso", bufs=1, space="PSUM") as pso,
    ):
        at8 = big.tile([128, JC * SH], F8, tag="at8")       # A.T, j on partitions
        h16 = big.tile([128, JC * D], F16, tag="h16")       # h, j on partitions
        wt = small.tile([3, N], F16, tag="wt")              # W'.T
        w4 = small.tile([128, JC * 4], F16, tag="w4")       # W'|ones (j on part)
        id16 = small.tile([128, 128], F16, tag="id16")
        scr = small.tile([128, 512], F16, tag="scr")        # warm-up source
        rN16 = small.tile([128, IC * H], F16, tag="rN16")   # rowsum/denom'
        rT16 = small.tile([3, SH], F16, tag="rT16")         # R'.T [k, i]

        # ---------------- loads ----------------
        # A.T (fp8, exact) first on the HWDGE/sync queue at full bandwidth;
        # h afterwards - it streams during the main loop. Small fp16 tiles on
        # the SWDGE/gpsimd queue so their generation doesn't delay A.T.
        a_r = a8_in.rearrange("(g jc p) i -> g p jc i", p=128, g=4)
        at8_v = at8[:].rearrange("p (g jc i) -> g p jc i", g=4, jc=JC // 4)
        for g in range(4):
            nc.sync.dma_start(out=at8_v[g], in_=a_r[g])
        h_r = h_in.rearrange("(g jc p) d -> g p jc d", p=128, g=8)
        h16_v = h16[:].rearrange("p (g jc d) -> g p jc d", g=8, jc=JC // 8)
        for g in range(8):
            nc.sync.dma_start(out=h16_v[g], in_=h_r[g])
        nc.gpsimd.dma_start(out=w4[:], in_=w4_in)
        nc.gpsimd.dma_start(out=id16[:], in_=id_in)
        nc.gpsimd.dma_start(out=wt[:], in_=wt_in)

        # ---- denominators chunked per A.T piece, PE warm-up filling gaps ----
        # psD2[p_i, ic*4+k] = sum_j A[i,j] W'[j,k];  k=3 gives rowsum.
        # The 4 ic accumulation regions share one PSUM bank (one group each,
        # started on the first piece, stopped on the last).
        nc.vector.memset(scr[:], 0.0)
        # warm the ACT table (LoadActFuncSet) off the critical path
        actw = small.tile([1, 2], F16, tag="actw")
        nc.scalar.copy(actw[:], scr[0:1, 0:2])
        n_warm = 0

        def warm(n):
            nonlocal n_warm
            for _ in range(n):
                pw = psa.tile([128, 512], F32, tag="scr", name=f"warm{n_warm}")
                nc.tensor.matmul(
                    pw[:], scr[:, 0:128], scr[:], start=True, stop=True
                )
                n_warm += 1

        psD2 = psd.tile([128, IC * 4], F32, tag="psd", name="psD2")
        warm(5)
        for g in range(4):
            for ic in range(IC):
                for jc in range(g * 8, (g + 1) * 8):
                    nc.tensor.matmul(
                        psD2[:, ic * 4:(ic + 1) * 4],
                        at8[:, jc * SH + ic * 128: jc * SH + ic * 128 + 128],
                        w4[:, jc * 4:(jc + 1) * 4],
                        start=(g == 0 and ic == 0 and jc == 0),
                        stop=(g == 3 and ic == IC - 1 and jc == 31),
                    )
            if g < 3:
                warm(7)

        # R' = rowsum/denom' in one DVE divide, transposed to [k, i] on PE
        psD2_v = psD2[:].rearrange("p (ic s) -> p ic s", s=4)
        if BCAST_DIV:
            nc.vector.tensor_tensor(
                rN16[:].rearrange("p (ic k) -> p ic k", k=H),
                psD2_v[:, :, 3:4].broadcast_to((128, IC, H)),
                psD2_v[:, :, 0:H],
                op=mybir.AluOpType.divide,
            )
        else:
            rN = small.tile([128, IC * H], F32, tag="rN")
            nc.vector.reciprocal(
                rN[:].rearrange("p (ic k) -> p ic k", k=H), psD2_v[:, :, 0:H]
            )
        psRT = psa.tile([3, SH], F16, tag="scr", name="psRT")
        for ic in range(IC):
            if not BCAST_DIV:
                # scalar (rowsum) read straight from PSUM; transpose chases
                # each mul so the PE ladder starts before the last mul
                nc.vector.tensor_scalar(
                    rN16[:, ic * H:(ic + 1) * H], rN[:, ic * H:(ic + 1) * H],
                    psD2[:, ic * 4 + 3: ic * 4 + 4], None, op0=mult,
                )
            nc.tensor.transpose(
                psRT[:, ic * 128:(ic + 1) * 128],
                rN16[:, ic * H:(ic + 1) * H],
                id16[:],
            )
        nc.vector.tensor_copy(rT16[:], psRT[:])

        # ---------------- main loop (ct software-pipelined 2 ahead) ----------
        psO = [
            pso.tile([128, D], F32, tag=f"psO{ic}", name=f"psO{ic}")
            for ic in range(IC)
        ]
        cts = [None] * JC

        def emit_ct(j):
            cts[j] = psa.tile([128, SH], F32, tag="scr", name=f"ct{j}")
            nc.tensor.matmul(
                cts[j][:],
                wt[0:3, j * 128:(j + 1) * 128],
                rT16[:],
                start=True,
                stop=True,
                tile_position=(0, 0),
            )

        emit_ct(0)
        emit_ct(1)
        for jc in range(JC):
            mt = mtp.tile([128, SH], F16, tag="mt", name=f"mt{jc}")
            nc.vector.tensor_tensor(
                mt[:], at8[:, jc * SH:(jc + 1) * SH], cts[jc][:], op=mult
            )
            if jc + 2 < JC:
                emit_ct(jc + 2)
            for ic in range(IC):
                nc.tensor.matmul(
                    psO[ic][:],
                    mt[:, ic * 128:(ic + 1) * 128],
                    h16[:, jc * D:(jc + 1) * D],
                    start=(jc == 0),
                    stop=(jc == JC - 1),
                )

        # ------- store: fp16 staging (host casts back), ACT/DVE copy split,
        # ------- DMAs split across HWDGE (sync) and SWDGE (gpsimd) queues ---
        out_r = out.rearrange("(ic p) d -> ic p d", p=128)
        for ic in range(IC):
            ot = osb.tile([128, D], F16, tag="ot", name=f"ot{ic}")
            if ic % 2 == 0:
                nc.scalar.copy(ot[:], psO[ic][:])
            else:
                nc.vector.tensor_copy(ot[:], psO[ic][:])
            # early tiles on the slower SWDGE path, late tiles on HWDGE
            if ic < 2:
                nc.gpsimd.dma_start(out=out_r[ic], in_=ot[:])
            else:
                nc.sync.dma_start(out=out_r[ic], in_=ot[:])


_CACHE = {}


def _build1():
    if "p1" in _CACHE:
        return _CACHE["p1"]
    nc = bacc.Bacc("TRN2", target_bir_lowering=False, debug=False,
                   num_devices=NCORES)
    hst_in = nc.dram_tensor("hst_in", [128, IC * DC * 128], F16,
                            kind="ExternalInput").ap()
    p_in = nc.dram_tensor("p_in", [128, DC * H], F16, kind="ExternalInput").ap()
    w_out = nc.dram_tensor("w_out", [128, IC * H], F16,
                           kind="ExternalOutput").ap()
    with tile.TileContext(nc) as tc:
        _body1(tc, hst_in, p_in, w_out)
    nc.compile()
    _CACHE["p1"] = nc
    return nc


def _build2():
    if "p2" in _CACHE:
        return _CACHE["p2"]
    nc = bacc.Bacc("TRN2", target_bir_lowering=False, debug=False,
                   num_devices=NCORES)
    a8_in = nc.dram_tensor("a8_in", [N, SH], F8, kind="ExternalInput").ap()
    h_in = nc.dram_tensor("h_in", [N, D], F16, kind="ExternalInput").ap()
    wt_in = nc.dram_tensor("wt_in", [3, N], F16, kind="ExternalInput").ap()
    w4_in = nc.dram_tensor("w4_in", [128, JC * 4], F16,
                           kind="ExternalInput").ap()
    id_in = nc.dram_tensor("id_in", [128, 128], F16, kind="ExternalInput").ap()
    out = nc.dram_tensor("out", [SH, D], F16, kind="ExternalOutput").ap()
    with tile.TileContext(nc) as tc:
        _body2(tc, a8_in, h_in, wt_in, w4_in, id_in, out)
    nc.compile()
    _CACHE["p2"] = nc
    return nc


def kernel(graph_info, h, P, _trace=False, _results_out=None):
    graph_info = np.ascontiguousarray(graph_info, dtype=np.float32)
    h = np.ascontiguousarray(h, dtype=np.float32)
    P = np.ascontiguousarray(P, dtype=np.float32)
    nc1 = _build1()
    nc2 = _build2()

    # host-side shard/layout prep (pure data movement + dtype casts)
    h16_full = h.astype(np.float16)
    p16_host = np.ascontiguousarray(
        P.astype(np.float16).reshape(DC, 128, H).transpose(1, 0, 2)
    ).reshape(128, DC * H)
    in1 = []
    for c in range(NCORES):
        hsT = h16_full[c * SH:(c + 1) * SH, :].T  # [D, SH]
        hst_host = np.ascontiguousarray(
            hsT.reshape(DC, 128, IC, 128).transpose(1, 2, 0, 3)
        ).reshape(128, IC * DC * 128)
        in1.append({"hst_in": hst_host, "p_in": p16_host})
    res1 = bass_utils.run_bass_kernel_spmd(
        nc1, in1, core_ids=list(range(NCORES)), trace=_trace
    )
    w_full = np.concatenate(
        [
            res1.results[c]["w_out"]
            .reshape(128, IC, H).transpose(1, 0, 2).reshape(SH, H)
            for c in range(NCORES)
        ],
        axis=0,
    )  # [N, 3] fp16, scaled by 2^-4

    wt_host = np.ascontiguousarray(w_full.T)  # [3, N]
    w4_host = np.ascontiguousarray(
        np.concatenate(
            [w_full.reshape(JC, 128, H).transpose(1, 0, 2),
             np.ones((128, JC, 1), np.float16)],
            axis=2,
        ).reshape(128, JC * 4)
    )
    id_host = np.eye(128, dtype=np.float16)

    in2 = [
        {
            "a8_in": np.ascontiguousarray(
                graph_info[c * SH:(c + 1) * SH, :].T
            ).astype(NP_F8),
            "h_in": h16_full,
            "wt_in": wt_host,
            "w4_in": w4_host,
            "id_in": id_host,
        }
        for c in range(NCORES)
    ]
    res2 = bass_utils.run_bass_kernel_spmd(
        nc2, in2, core_ids=list(range(NCORES)), trace=_trace
    )
    if _results_out is not None:
        _results_out.extend([res1, res2])
    return np.concatenate(
        [res2.results[c]["out"].astype(np.float32) for c in range(NCORES)],
        axis=0,
    )


# revision 40
# speedup vs baseline: 1.4140x; 1.0056x over previous
"""GAT-style attention (gnn_message_passing) Trainium2 kernel, 8-core row-parallel.

Math (algebraically identical to the reference masked-softmax attention):
  E = relu(h @ P)                 [N,3]
  W' = max(exp(E - 4ln2), 1/16)   (= exp(relu(E))/16, fp16-safe range)
  denom'[i,k] = sum_j A[i,j] W'[j,k]   (k=3 slot sums ones -> rowsum[i])
  R'[i,k] = rowsum[i] / denom'[i,k]
  ct[j,i]  = sum_k W'[j,k] R'[i,k] = rowsum[i] * C[i,j]
  out[i,:] = sum_j A[i,j] ct[j,i] h[j,:]

Two SPMD programs (collectives unavailable on this runtime path; the tiny
[4096,3] W matrix crosses cores via a host gather between programs):
  P1 (per core): W'-shard [512,3] from host-transposed h-shard. The E matmuls
      use h.T as the *stationary* operand so each streams only 3 columns.
  host: concat the 8 W'-shards; pack W'.T, W'|ones; cast A-shard.T to fp8
      (binary, exact) and h to fp16  (pure data movement / layout).
  P2 (per core): denominators via at8-stationary matmuls ([128,4] outputs,
      accumulated in one PSUM bank); rowsum folded into R' so no final scale;
      C.T tiles via PE (K=3); mask-multiply on DVE; main (A*C).T @ h on PE
      with h streaming during the loop. PE warm-up matmuls run during the
      A.T load so the main loop starts at full clock.
"""

import numpy as np
import ml_dtypes

import concourse.bass as bass
import concourse.mybir as mybir
import concourse.tile as tile
from concourse import bacc
from concourse import bass_utils

N = 4096
D = 512
H = 3
NCORES = 8
SH = N // NCORES          # 512 output rows per core
JC = N // 128             # 32 j-chunks
IC = SH // 128            # 4 i-chunks
DC = D // 128             # 4 d-chunks
F8 = mybir.dt.float8e4
F16 = mybir.dt.float16
F32 = mybir.dt.float32
LN2x4 = float(4.0 * np.log(2.0))   # W scaled by 2^-4 to stay in fp16 range
N_WARMUP = 24                      # PE warm-up matmuls during the A.T load
NP_F8 = ml_dtypes.float8_e4m3
BCAST_DIV = False                  # stride-0 divide (walrus-compile suspect)


def _body1(tc, hst_in, p_in, w_out):
    """P1: W'-shard [SH,3] from hst [128, IC*DC*128] (h-shard.T, jc-major:
    hst[:, jc, dc, :] = h.T d-chunk dc for j-chunk jc). Loaded in 4 jc
    pieces; exp/max pipeline behind the pieces."""
    nc = tc.nc
    with (
        tc.tile_pool(name="sb1", bufs=1) as sb,
        tc.tile_pool(name="ps1", bufs=1, space="PSUM") as ps,
    ):
        hst = sb.tile([128, IC * DC * 128], F16, tag="hst")
        p16 = sb.tile([128, DC * H], F16, tag="p16")
        wsE = sb.tile([128, IC * H], F16, tag="wsE")
        ebias = sb.tile([128, 1], F32, tag="ebias")
        nc.vector.memset(ebias[:], -LN2x4)
        hst_v = hst[:].rearrange("p (g x) -> g p x", g=2)
        hin_v = hst_in.rearrange("p (g x) -> g p x", g=2)
        for g in range(2):
            nc.sync.dma_start(out=hst_v[g], in_=hin_v[g])
        nc.gpsimd.dma_start(out=p16[:], in_=p_in)

        # one PSUM tile spanning 4 banks: E group per jc, single exp at the end
        psE = ps.tile([128, IC * 512], F32, tag="psE", name="psE")
        for jc in range(IC):
            for dc in range(DC):
                nc.tensor.matmul(
                    psE[:, jc * 512: jc * 512 + H],
                    hst[:, (jc * DC + dc) * 128: (jc * DC + dc + 1) * 128],
                    p16[:, dc * H:(dc + 1) * H],
                    start=(dc == 0),
                    stop=(dc == DC - 1),
                )
        nc.scalar.activation(
            wsE[:].rearrange("p (jc k) -> p jc k", k=H),
            psE[:].rearrange("p (jc x) -> p jc x", x=512)[:, :, 0:H],
            mybir.ActivationFunctionType.Exp,
            bias=ebias[:], scale=1.0,
        )
        nc.vector.tensor_scalar_max(wsE[:], wsE[:], 0.0625)
        nc.sync.dma_start(out=w_out, in_=wsE[:])


def _body2(tc, a8_in, h_in, wt_in, w4_in, id_in, out):
    """P2: the heavy pipeline. wt_in [3,N] / w4_in [128,JC*4] are host layouts
    of the device-computed (scaled) W' from P1; a8_in is A-shard.T in fp8."""
    nc = tc.nc
    mult = mybir.AluOpType.mult

    with (
        tc.tile_pool(name="big", bufs=1) as big,
        tc.tile_pool(name="small", bufs=1) as small,
        tc.tile_pool(name="mtp", bufs=4) as mtp,
        tc.tile_pool(name="osb", bufs=4) as osb,
        tc.tile_pool(name="psa", bufs=3, space="PSUM") as psa,
        tc.tile_pool(name="psd", bufs=1, space="PSUM") as psd,
        tc.tile_pool(name="pso", bufs=1, space="PSUM") as pso,
    ):
        at8 = big.tile([128, JC * SH], F8, tag="at8")       # A.T, j on partitions
        h16 = big.tile([128, JC * D], F16, tag="h16")       # h, j on partitions
        wt = small.tile([3, N], F16, tag="wt")              # W'.T
        w4 = small.tile([128, JC * 4], F16, tag="w4")       # W'|ones (j on part)
        id16 = small.tile([128, 128], F16, tag="id16")
        scr = small.tile([128, 512], F16, tag="scr")        # warm-up source
        rN16 = small.tile([128, IC * H], F16, tag="rN16")   # rowsum/denom'
        rT16 = small.tile([3, SH], F16, tag="rT16")         # R'.T [k, i]

        # ---------------- loads ----------------
        # A.T (fp8, exact) first on the HWDGE/sync queue at full bandwidth;
        # h afterwards - it streams during the main loop. Small fp16 tiles on
        # the SWDGE/gpsimd queue so their generation doesn't delay A.T.
        a_r = a8_in.rearrange("(g jc p) i -> g p jc i", p=128, g=4)
        at8_v = at8[:].rearrange("p (g jc i) -> g p jc i", g=4, jc=JC // 4)
        for g in range(4):
            nc.sync.dma_start(out=at8_v[g], in_=a_r[g])
        h_r = h_in.rearrange("(g jc p) d -> g p jc d", p=128, g=8)
        h16_v = h16[:].rearrange("p (g jc d) -> g p jc d", g=8, jc=JC // 8)
        for g in range(8):
            nc.sync.dma_start(out=h16_v[g], in_=h_r[g])
        nc.gpsimd.dma_start(out=w4[:], in_=w4_in)
        nc.gpsimd.dma_start(out=id16[:], in_=id_in)
        nc.gpsimd.dma_start(out=wt[:], in_=wt_in)

        # ---- denominators chunked per A.T piece, PE warm-up filling gaps ----
        # psD2[p_i, ic*4+k] = sum_j A[i,j] W'[j,k];  k=3 gives rowsum.
        # The 4 ic accumulation regions share one PSUM bank (one group each,
        # started on the first piece, stopped on the last).
        nc.vector.memset(scr[:], 0.0)
        # warm the ACT table (LoadActFuncSet) off the critical path
        actw = small.tile([1, 2], F16, tag="actw")
        nc.scalar.copy(actw[:], scr[0:1, 0:2])
        n_warm = 0

        def warm(n):
            nonlocal n_warm
            for _ in range(n):
                pw = psa.tile([128, 512], F32, tag="scr", name=f"warm{n_warm}")
                nc.tensor.matmul(
                    pw[:], scr[:, 0:128], scr[:], start=True, stop=True
                )
                n_warm += 1

        psD2 = psd.tile([128, IC * 4], F32, tag="psd", name="psD2")
        warm(5)
        for g in range(4):
            for ic in range(IC):
                for jc in range(g * 8, (g + 1) * 8):
                    nc.tensor.matmul(
                        psD2[:, ic * 4:(ic + 1) * 4],
                        at8[:, jc * SH + ic * 128: jc * SH + ic * 128 + 128],
                        w4[:, jc * 4:(jc + 1) * 4],
                        start=(g == 0 and ic == 0 and jc == 0),
                        stop=(g == 3 and ic == IC - 1 and jc == 31),
                    )
            if g < 3:
                warm(7 if g < 2 else 8)

        # R' = rowsum/denom' in one DVE divide, transposed to [k, i] on PE
        psD2_v = psD2[:].rearrange("p (ic s) -> p ic s", s=4)
        if BCAST_DIV:
            nc.vector.tensor_tensor(
                rN16[:].rearrange("p (ic k) -> p ic k", k=H),
                psD2_v[:, :, 3:4].broadcast_to((128, IC, H)),
                psD2_v[:, :, 0:H],
                op=mybir.AluOpType.divide,
            )
        else:
            rN = small.tile([128, IC * H], F32, tag="rN")
            nc.vector.reciprocal(
                rN[:].rearrange("p (ic k) -> p ic k", k=H), psD2_v[:, :, 0:H]
            )
        psRT = psa.tile([3, SH], F16, tag="scr", name="psRT")
        for ic in range(IC):
            if not BCAST_DIV:
                # scalar (rowsum) read straight from PSUM; transpose chases
                # each mul so the PE ladder starts before the last mul
                nc.vector.tensor_scalar(
                    rN16[:, ic * H:(ic + 1) * H], rN[:, ic * H:(ic + 1) * H],
                    psD2[:, ic * 4 + 3: ic * 4 + 4], None, op0=mult,
                )
            nc.tensor.transpose(
                psRT[:, ic * 128:(ic + 1) * 128],
                rN16[:, ic * H:(ic + 1) * H],
                id16[:],
            )
        nc.vector.tensor_copy(rT16[:], psRT[:])

        # ---------------- main loop (ct software-pipelined 2 ahead) ----------
        psO = [
            pso.tile([128, D], F32, tag=f"psO{ic}", name=f"psO{ic}")
            for ic in range(IC)
        ]
        cts = [None] * JC

        def emit_ct(j):
            cts[j] = psa.tile([128, SH], F32, tag="scr", name=f"ct{j}")
            nc.tensor.matmul(
                cts[j][:],
                wt[0:3, j * 128:(j + 1) * 128],
                rT16[:],
                start=True,
                stop=True,
                tile_position=(0, 0),
            )

        emit_ct(0)
        emit_ct(1)
        for jc in range(JC):
            mt = mtp.tile([128, SH], F16, tag="mt", name=f"mt{jc}")
            nc.vector.tensor_tensor(
                mt[:], at8[:, jc * SH:(jc + 1) * SH], cts[jc][:], op=mult
            )
            if jc + 2 < JC:
                emit_ct(jc + 2)
            for ic in range(IC):
                nc.tensor.matmul(
                    psO[ic][:],
                    mt[:, ic * 128:(ic + 1) * 128],
                    h16[:, jc * D:(jc + 1) * D],
                    start=(jc == 0),
                    stop=(jc == JC - 1),
                )

        # ------- store: fp16 staging (host casts back), ACT/DVE copy split,
        # ------- DMAs split across HWDGE (sync) and SWDGE (gpsimd) queues ---
        out_r = out.rearrange("(ic p) d -> ic p d", p=128)
        for ic in range(IC):
            ot = osb.tile([128, D], F16, tag="ot", name=f"ot{ic}")
            if ic % 2 == 0:
                nc.scalar.copy(ot[:], psO[ic][:])
            else:
                nc.vector.tensor_copy(ot[:], psO[ic][:])
            # first tile on the slower SWDGE path, rest on HWDGE
            if ic == 0:
                nc.gpsimd.dma_start(out=out_r[ic], in_=ot[:])
            else:
                nc.sync.dma_start(out=out_r[ic], in_=ot[:])


_CACHE = {}


def _build1():
    if "p1" in _CACHE:
        return _CACHE["p1"]
    nc = bacc.Bacc("TRN2", target_bir_lowering=False, debug=False,
                   num_devices=NCORES)
    hst_in = nc.dram_tensor("hst_in", [128, IC * DC * 128], F16,
                            kind="ExternalInput").ap()
    p_in = nc.dram_tensor("p_in", [128, DC * H], F16, kind="ExternalInput").ap()
    w_out = nc.dram_tensor("w_out", [128, IC * H], F16,
                           kind="ExternalOutput").ap()
    with tile.TileContext(nc) as tc:
        _body1(tc, hst_in, p_in, w_out)
    nc.compile()
    _CACHE["p1"] = nc
    return nc


def _build2():
    if "p2" in _CACHE:
        return _CACHE["p2"]
    nc = bacc.Bacc("TRN2", target_bir_lowering=False, debug=False,
                   num_devices=NCORES)
    a8_in = nc.dram_tensor("a8_in", [N, SH], F8, kind="ExternalInput").ap()
    h_in = nc.dram_tensor("h_in", [N, D], F16, kind="ExternalInput").ap()
    wt_in = nc.dram_tensor("wt_in", [3, N], F16, kind="ExternalInput").ap()
    w4_in = nc.dram_tensor("w4_in", [128, JC * 4], F16,
                           kind="ExternalInput").ap()
    id_in = nc.dram_tensor("id_in", [128, 128], F16, kind="ExternalInput").ap()
    out = nc.dram_tensor("out", [SH, D], F16, kind="ExternalOutput").ap()
    with tile.TileContext(nc) as tc:
        _body2(tc, a8_in, h_in, wt_in, w4_in, id_in, out)
    nc.compile()
    _CACHE["p2"] = nc
    return nc


def kernel(graph_info, h, P, _trace=False, _results_out=None):
    graph_info = np.ascontiguousarray(graph_info, dtype=np.float32)
    h = np.ascontiguousarray(h, dtype=np.float32)
    P = np.ascontiguousarray(P, dtype=np.float32)
    nc1 = _build1()
    nc2 = _build2()

    # host-side shard/layout prep (pure data movement + dtype casts)
    h16_full = h.astype(np.float16)
    p16_host = np.ascontiguousarray(
        P.astype(np.float16).reshape(DC, 128, H).transpose(1, 0, 2)
    ).reshape(128, DC * H)
    in1 = []
    for c in range(NCORES):
        hsT = h16_full[c * SH:(c + 1) * SH, :].T  # [D, SH]
        hst_host = np.ascontiguousarray(
            hsT.reshape(DC, 128, IC, 128).transpose(1, 2, 0, 3)
        ).reshape(128, IC * DC * 128)
        in1.append({"hst_in": hst_host, "p_in": p16_host})
    res1 = bass_utils.run_bass_kernel_spmd(
        nc1, in1, core_ids=list(range(NCORES)), trace=_trace
    )
    w_full = np.concatenate(
        [
            res1.results[c]["w_out"]
            .reshape(128, IC, H).transpose(1, 0, 2).reshape(SH, H)
            for c in range(NCORES)
        ],
        axis=0,
    )  # [N, 3] fp16, scaled by 2^-4

    wt_host = np.ascontiguousarray(w_full.T)  # [3, N]
    w4_host = np.ascontiguousarray(
        np.concatenate(
            [w_full.reshape(JC, 128, H).transpose(1, 0, 2),
             np.ones((128, JC, 1), np.float16)],
            axis=2,
        ).reshape(128, JC * 4)
    )
    id_host = np.eye(128, dtype=np.float16)

    in2 = [
        {
            "a8_in": np.ascontiguousarray(
                graph_info[c * SH:(c + 1) * SH, :].T
            ).astype(NP_F8),
            "h_in": h16_full,
            "wt_in": wt_host,
            "w4_in": w4_host,
            "id_in": id_host,
        }
        for c in range(NCORES)
    ]
    res2 = bass_utils.run_bass_kernel_spmd(
        nc2, in2, core_ids=list(range(NCORES)), trace=_trace
    )
    if _results_out is not None:
        _results_out.extend([res1, res2])
    return np.concatenate(
        [res2.results[c]["out"].astype(np.float32) for c in range(NCORES)],
        axis=0,
    )
